# revision 5
# baseline (speedup 1.0000x reference)
"""Trainium2 Bass kernel for nn_MergeHead (dynamic-conv head + dice loss).

Sharding: 8 cores = 2 batches x 4 row-blocks of 16 image rows each.
Each core computes, for its batch b and rows [r0, r0+16):
  - h0 = relu(conv3x3(feats))           (rows r0-1 .. r0+17, via halo)
  - h1 = relu(conv3x3(h0))              (rows r0 .. r0+16)
  - kernels = 1x1(h1) -> kw^T [256 x 1024], kb [1 x 1024]
  - f = 1x1(feats_full) [256 x 4096]
  - logits[p, hw] = kw^T.T @ f + kb  (p in core's 1024 rows)
  - output: max(logits + Mneg, -1e8)  (Mneg = -2^30 at masked cols)
  - loss partial: s[p] = sum_hw sin(2*atan(sigma(x_loss)/sqrt(1.002)))
    where x_loss = logits + kb + Mneg + 2^30*(eq-1)  (eq = label match via
    one-hot K=16 matmul).  Identity: 2u/(u^2+1.002) =
    (1/sqrt(1.002)) * sin(2*atan(u/sqrt(1.002))), u = sigmoid(x).
Host combines: loss = mean_b (grid^2 - S_b) / (grid^2 + 1e-5).
"""
import math

import numpy as np

import concourse.bass as bass
import concourse.bacc as bacc
import concourse.tile as tile
from concourse import mybir
from concourse.bass_utils import run_bass_kernel_spmd

dt = mybir.dt
F32 = dt.float32
F32R = dt.float32r
BF16 = dt.bfloat16
AF = mybir.ActivationFunctionType
ALU = mybir.AluOpType

B, C, H, W = 2, 256, 64, 64
HW = H * W            # 4096
P_CORE = 1024         # p rows per core
ROWS = 16             # image rows per core
N_CORES = 8
BIG = float(2 ** 30)
NEG_INF = -1e8
A_EPS = math.sqrt(1.002)

_NC = None


def _emit(nc):
    # ---- DRAM I/O ----
    feats2_d = nc.dram_tensor("feats2", [128, 2, HW], F32, kind="ExternalInput").ap()
    fhalo_d = nc.dram_tensor("fhalo", [128, 2, 20, 66], F32, kind="ExternalInput").ap()
    zrow_d = nc.dram_tensor("zrow", [128, 66], F32, kind="ExternalInput").ap()
    wp0_d = nc.dram_tensor("wp0", [128, 9, 2, 2, 128], F32, kind="ExternalInput").ap()
    wp1_d = nc.dram_tensor("wp1", [128, 9, 2, 2, 128], F32, kind="ExternalInput").ap()
    wk_d = nc.dram_tensor("wk", [128, 2, 257], F32, kind="ExternalInput").ap()
    wf_d = nc.dram_tensor("wf", [128, 2, 256], F32, kind="ExternalInput").ap()
    biases_d = nc.dram_tensor("biases", [128, 11], F32, kind="ExternalInput").ap()
    lhs_ex_d = nc.dram_tensor("lhs_ex", [17, P_CORE], F32, kind="ExternalInput").ap()
    rhs_ex_d = nc.dram_tensor("rhs_ex", [19, HW], F32, kind="ExternalInput").ap()

    out_d = nc.dram_tensor("out", [P_CORE, HW], F32, kind="ExternalOutput").ap()
    s_out_d = nc.dram_tensor("s_out", [128, 32], F32, kind="ExternalOutput").ap()

    with tile.TileContext(nc) as tc:
        with tc.tile_pool(name="consts", bufs=1) as consts:
            wk_sb = consts.tile([128, 2, 257], F32R, tag="wk")
            wf_sb = consts.tile([128, 2, 256], F32R, tag="wf")
            biases = consts.tile([128, 11], F32, tag="biases")
            rhs_sb = consts.tile([49, HW], F32R, tag="rhs_sb")
            extra_sb = consts.tile([49, P_CORE], F32R, tag="extra_sb")
            kern = consts.tile([128, 2, P_CORE], F32R, tag="kern")
            f_sb = consts.tile([128, 2, HW], F32R, tag="f_sb")
            h1 = consts.tile([128, 2, ROWS * 64], F32R, tag="h1")
            s1cols = consts.tile([128, 32], F32, tag="s1cols")

            nc.gpsimd.dma_start(out=wk_sb[:], in_=wk_d)
            nc.gpsimd.dma_start(out=wf_sb[:], in_=wf_d)
            nc.sync.dma_start(out=biases[:], in_=biases_d)
            nc.gpsimd.dma_start(out=rhs_sb[0:2, :], in_=rhs_ex_d[0:2, :])
            nc.gpsimd.dma_start(out=rhs_sb[32:49, :], in_=rhs_ex_d[2:19, :])
            nc.gpsimd.dma_start(out=extra_sb[32:49, :], in_=lhs_ex_d)
            nc.gpsimd.dma_start(out=extra_sb[1:2, :], in_=lhs_ex_d[0:1, :])

            # ---------- conv phase (scoped pools; freed before loss phase) ----
            with tc.tile_pool(name="convp", bufs=1) as convp, \
                 tc.tile_pool(name="fblk", bufs=3) as fblkp, \
                 tc.tile_pool(name="cpsum", bufs=2, space="PSUM") as cps, \
                 tc.tile_pool(name="fpsum", bufs=2, space="PSUM") as fps:
                wp0 = convp.tile([128, 9, 2, 2, 128], F32R, tag="wp0")
                wp1 = convp.tile([128, 9, 2, 2, 128], F32R, tag="wp1")
                fpad = convp.tile([128, 2, 20, 66], F32R, tag="fpad")
                h0p = convp.tile([128, 2, 18, 66], F32R, tag="h0p")

                nc.gpsimd.dma_start(out=wp0[:], in_=wp0_d)
                nc.gpsimd.dma_start(out=wp1[:], in_=wp1_d)
                nc.gpsimd.dma_start(out=fpad[:], in_=fhalo_d)
                nc.gpsimd.dma_start(out=h0p[:, :, :, 0:1], in_=zrow_d[:, 0:36])
                nc.gpsimd.dma_start(out=h0p[:, :, :, 65:66], in_=zrow_d[:, 0:36])

                # conv1: fpad rows (image r0-2..r0+17) -> h0p rows (r0-1..r0+16)
                for ot in range(2):
                    for (R, nb) in ((0, 8), (8, 8), (16, 2)):
                        ps = cps.tile([128, 512], F32, tag="conv")
                        n = nb * 64
                        for kt in range(2):
                            for tap in range(9):
                                dy, dx = tap // 3 - 1, tap % 3 - 1
                                nc.tensor.matmul(
                                    ps[:, :n],
                                    wp0[:, tap, kt, ot, :],
                                    fpad[:, kt, R + dy + 1:R + dy + 1 + nb,
                                         dx + 1:dx + 1 + 64],
                                    start=(kt == 0 and tap == 0),
                                    stop=(kt == 1 and tap == 8))
                        nc.vector.tensor_scalar(
                            out=h0p[:, ot, R:R + nb, 1:65], in0=ps[:, :n],
                            scalar1=biases[:, ot:ot + 1], scalar2=0.0,
                            op0=ALU.add, op1=ALU.max)

                # zero out-of-image h0 pad rows (row-kill: 0.0 on boundary cores)
                for kt in range(2):
                    nc.vector.tensor_scalar(
                        out=h0p[:, kt, 0, :], in0=h0p[:, kt, 0, :],
                        scalar1=biases[:, 9:10], scalar2=None, op0=ALU.mult)
                    nc.vector.tensor_scalar(
                        out=h0p[:, kt, 17, :], in0=h0p[:, kt, 17, :],
                        scalar1=biases[:, 10:11], scalar2=None, op0=ALU.mult)

                # conv2: h0p rows (r0-1..r0+16) -> h1 rows (r0..r0+15)
                for ot in range(2):
                    for R in (0, 8):
                        ps = cps.tile([128, 512], F32, tag="conv")
                        for kt in range(2):
                            for tap in range(9):
                                dy, dx = tap // 3 - 1, tap % 3 - 1
                                nc.tensor.matmul(
                                    ps[:],
                                    wp1[:, tap, kt, ot, :],
                                    h0p[:, kt, R + dy + 1:R + dy + 1 + 8,
                                        dx + 1:dx + 1 + 64],
                                    start=(kt == 0 and tap == 0),
                                    stop=(kt == 1 and tap == 8))
                        nc.vector.tensor_scalar(
                            out=h1[:, ot, R * 64:R * 64 + 512], in0=ps[:],
                            scalar1=biases[:, 2 + ot:3 + ot], scalar2=0.0,
                            op0=ALU.add, op1=ALU.max)

                # kernels 1x1: h1 -> kern [c, p] and kb row -> extra_sb[0]
                for ot in range(2):
                    for pb in range(2):
                        ps = cps.tile([128, 512], F32, tag="conv")
                        for kt in range(2):
                            nc.tensor.matmul(
                                ps[:], wk_sb[:, kt, ot * 128:(ot + 1) * 128],
                                h1[:, kt, pb * 512:(pb + 1) * 512],
                                start=(kt == 0), stop=(kt == 1))
                        nc.scalar.activation(
                            kern[:, ot, pb * 512:(pb + 1) * 512], ps[:],
                            AF.Identity, bias=biases[:, 4 + ot:5 + ot])
                for pb in range(2):
                    ps = cps.tile([1, 512], F32, tag="kb")
                    for kt in range(2):
                        nc.tensor.matmul(
                            ps[:], wk_sb[:, kt, 256:257],
                            h1[:, kt, pb * 512:(pb + 1) * 512],
                            start=(kt == 0), stop=(kt == 1))
                    nc.scalar.activation(
                        extra_sb[0:1, pb * 512:(pb + 1) * 512], ps[:],
                        AF.Identity, bias=biases[0:1, 8:9])

                # f 1x1: feats (streamed) -> f_sb [c, hw]
                for fb in range(4):
                    fblk = fblkp.tile([128, 2, 1024], F32R, tag="fblk")
                    nc.gpsimd.dma_start(
                        out=fblk[:], in_=feats2_d[:, :, fb * 1024:(fb + 1) * 1024])
                    for ot in range(2):
                        ps = fps.tile([128, 1024], F32, tag="fps")
                        for half in range(2):
                            for kt in range(2):
                                nc.tensor.matmul(
                                    ps[:, half * 512:(half + 1) * 512],
                                    wf_sb[:, kt, ot * 128:(ot + 1) * 128],
                                    fblk[:, kt, half * 512:(half + 1) * 512],
                                    start=(kt == 0), stop=(kt == 1))
                        nc.scalar.activation(
                            f_sb[:, ot, fb * 1024:(fb + 1) * 1024], ps[:],
                            AF.Identity, bias=biases[:, 6 + ot:7 + ot])

            # ---------- big loop: logits + loss ----------
            with tc.tile_pool(name="bpsum", bufs=3, space="PSUM") as bps, \
                 tc.tile_pool(name="upool", bufs=2) as upool, \
                 tc.tile_pool(name="atpool", bufs=8) as atpool, \
                 tc.tile_pool(name="lrow", bufs=3) as lrow, \
                 tc.tile_pool(name="soutp", bufs=2) as soutp:
                at_tiles = []
                for pt in range(8):
                    at_t = atpool.tile([128, HW], BF16, tag="at")
                    at_tiles.append(at_t)
                    for jb in range(4):
                        ps = bps.tile([128, 1024], F32, tag="big")
                        for half in range(2):
                            sl = slice(half * 512, (half + 1) * 512)
                            col = slice(jb * 1024 + half * 512,
                                        jb * 1024 + half * 512 + 512)
                            for kt in range(2):
                                nc.tensor.matmul(
                                    ps[:, sl],
                                    kern[:, kt, pt * 128:(pt + 1) * 128],
                                    f_sb[:, kt, col],
                                    start=(kt == 0), stop=False)
                            # kb + Mneg (rows 0:2 of extras)
                            nc.tensor.matmul(
                                ps[:, sl],
                                extra_sb[0:2, pt * 128:(pt + 1) * 128],
                                rhs_sb[0:2, col],
                                start=False, stop=True)
                        # output evict: max(logits + kb + Mneg, -1e8)
                        lr = lrow.tile([128, 1024], F32, tag="lr")
                        nc.vector.tensor_scalar(
                            out=lr[:], in0=ps[:], scalar1=NEG_INF, scalar2=None,
                            op0=ALU.max)
                        nc.sync.dma_start(
                            out=out_d[pt * 128:(pt + 1) * 128,
                                      jb * 1024:(jb + 1) * 1024],
                            in_=lr[:])
                        # label-mask extras: +2^30*(eq-1)
                        for half in range(2):
                            sl = slice(half * 512, (half + 1) * 512)
                            col = slice(jb * 1024 + half * 512,
                                        jb * 1024 + half * 512 + 512)
                            nc.tensor.matmul(
                                ps[:, sl],
                                extra_sb[32:49, pt * 128:(pt + 1) * 128],
                                rhs_sb[32:49, col],
                                start=False, stop=True, skip_group_check=True)
                        u_t = upool.tile([128, 1024], F32, tag="u")
                        nc.scalar.activation(u_t[:], ps[:], AF.Sigmoid)
                        nc.scalar.activation(
                            at_t[:, jb * 1024:(jb + 1) * 1024], u_t[:],
                            AF.Arctan, scale=1.0 / A_EPS)

                # sin phase (single ACT table switch)
                for pt in range(8):
                    for jb in range(4):
                        so = soutp.tile([128, 1024], BF16, tag="so")
                        nc.scalar.activation(
                            so[:], at_tiles[pt][:, jb * 1024:(jb + 1) * 1024],
                            AF.Sin, scale=2.0,
                            accum_out=s1cols[:, pt * 4 + jb:pt * 4 + jb + 1])
                nc.sync.dma_start(out=s_out_d, in_=s1cols[:])


def _build():
    global _NC
    if _NC is not None:
        return _NC
    nc = bacc.Bacc("TRN2", target_bir_lowering=False, debug=False,
                   num_devices=N_CORES)
    _emit(nc)
    nc.compile()
    _NC = nc
    return nc


def _prep_core(inputs, c):
    b, q = c // 4, c % 4
    r0 = q * ROWS
    p0 = q * P_CORE
    feats = np.ascontiguousarray(inputs["feats"][b], dtype=np.float32)  # [256,64,64]
    fflat = feats.reshape(C, HW)
    masks = np.asarray(inputs["masks"][b], dtype=np.float32).reshape(HW)
    layouts = np.asarray(inputs["layouts"][b]).reshape(HW)

    feats2 = fflat.reshape(2, 128, HW).transpose(1, 0, 2)  # [128,2,4096]

    halo = np.zeros((C, 20, 66), np.float32)
    lo, hi = max(r0 - 2, 0), min(r0 + 18, H)
    halo[:, lo - (r0 - 2):hi - (r0 - 2), 1:65] = feats[:, lo:hi, :]
    fhalo = halo.reshape(2, 128, 20, 66).transpose(1, 0, 2, 3)

    def prep_w33(w):  # [o, i, 3, 3] -> [128, 9, 2, 2, 128]
        t = np.asarray(w, np.float32).transpose(2, 3, 1, 0).reshape(9, 2, 128, 2, 128)
        return np.ascontiguousarray(t.transpose(2, 0, 1, 3, 4))

    wk = np.asarray(inputs["w_kernel"], np.float32).reshape(257, 256)
    wk_t = np.ascontiguousarray(wk.T.reshape(2, 128, 257).transpose(1, 0, 2))
    wf = np.asarray(inputs["w_feats"], np.float32).reshape(256, 256)
    wf_t = np.ascontiguousarray(wf.T.reshape(2, 128, 256).transpose(1, 0, 2))

    bz = np.zeros((128, 11), np.float32)
    bz[:, 0] = inputs["b_pre0"][:128]
    bz[:, 1] = inputs["b_pre0"][128:]
    bz[:, 2] = inputs["b_pre1"][:128]
    bz[:, 3] = inputs["b_pre1"][128:]
    bz[:, 4] = inputs["b_kernel"][:128]
    bz[:, 5] = inputs["b_kernel"][128:256]
    bz[:, 6] = inputs["b_feats"][:128]
    bz[:, 7] = inputs["b_feats"][128:]
    bz[:, 8] = inputs["b_kernel"][256]
    bz[:, 9] = 0.0 if r0 == 0 else 1.0
    bz[:, 10] = 0.0 if r0 + ROWS == H else 1.0

    oh_hw = (layouts[None, :] == np.arange(16)[:, None]).astype(np.float32)
    lhs_ex = np.ones((17, P_CORE), np.float32)
    lhs_ex[1:17] = oh_hw[:, p0:p0 + P_CORE]
    rhs_ex = np.zeros((19, HW), np.float32)
    rhs_ex[0] = 1.0
    rhs_ex[1] = np.where(masks > 0, 0.0, -BIG)   # Mneg
    rhs_ex[2] = -BIG
    rhs_ex[3:19] = BIG * oh_hw

    return {
        "feats2": np.ascontiguousarray(feats2),
        "fhalo": np.ascontiguousarray(fhalo),
        "wp0": prep_w33(inputs["w_pre0"]),
        "wp1": prep_w33(inputs["w_pre1"]),
        "wk": wk_t, "wf": wf_t, "biases": bz,
        "lhs_ex": lhs_ex, "rhs_ex": rhs_ex,
        "zrow": np.zeros((128, 66), np.float32),
    }


def prep_in_maps(inputs):
    inputs = {k: np.asarray(v) for k, v in inputs.items()}
    return [_prep_core(inputs, c) for c in range(N_CORES)]


def postprocess(results, inputs):
    logits = np.empty((B, HW, HW), np.float32)
    S = np.zeros(B, np.float64)
    masks = np.asarray(inputs["masks"], dtype=np.float32).reshape(B, HW)
    for c in range(N_CORES):
        b, q = c // 4, c % 4
        p0 = q * P_CORE
        logits[b, p0:p0 + P_CORE, :] = results[c]["out"]
        s = results[c]["s_out"].astype(np.float64)          # [128, 32]
        s_vec = s.reshape(128, 8, 4).sum(-1).T.reshape(P_CORE)  # p = pt*128+i
        S[b] += (masks[b, p0:p0 + P_CORE].astype(np.float64) * s_vec).sum()
    S /= A_EPS
    grid = masks.sum(1).astype(np.float64)
    loss = np.float32(((grid * grid - S) / (grid * grid + 1e-5)).mean())
    return logits.reshape(B, HW, H, W), loss


def run(inputs, trace=False):
    nc = _build()
    in_maps = prep_in_maps(inputs)
    res = run_bass_kernel_spmd(nc, in_maps, list(range(N_CORES)), trace=trace)
    logits, loss = postprocess(res.results, inputs)
    return logits, loss, res


def kernel(**inputs):
    logits, loss, _ = run(inputs)
    return logits, loss


# revision 6
# speedup vs baseline: 1.1299x; 1.1299x over previous
"""Trainium2 Bass kernel for nn_MergeHead (dynamic-conv head + dice loss).

Sharding: 8 cores = 2 batches x 4 row-blocks of 16 image rows each.
Each core computes, for its batch b and rows [r0, r0+16):
  - h0 = relu(conv3x3(feats))           (rows r0-1 .. r0+17, via halo)
  - h1 = relu(conv3x3(h0))              (rows r0 .. r0+16)
  - kernels = 1x1(h1) -> kw^T [256 x 1024], kb [1 x 1024]
  - f = 1x1(feats_full) [256 x 4096]
  - logits[p, hw] = kw^T.T @ f + kb  (p in core's 1024 rows)
  - output: max(logits + Mneg, -1e8)  (Mneg = -2^30 at masked cols)
  - loss partial: s[p] = sum_hw sin(2*atan(sigma(x_loss)/sqrt(1.002)))
    where x_loss = logits + kb + Mneg + 2^30*(eq-1)  (eq = label match via
    one-hot K=16 matmul).  Identity: 2u/(u^2+1.002) =
    (1/sqrt(1.002)) * sin(2*atan(u/sqrt(1.002))), u = sigmoid(x).
Host combines: loss = mean_b (grid^2 - S_b) / (grid^2 + 1e-5).
"""
import math

import numpy as np

import concourse.bass as bass
import concourse.bacc as bacc
import concourse.tile as tile
from concourse import mybir
from concourse.bass_utils import run_bass_kernel_spmd

dt = mybir.dt
F32 = dt.float32
F32R = dt.float32r
BF16 = dt.bfloat16
AF = mybir.ActivationFunctionType
ALU = mybir.AluOpType

B, C, H, W = 2, 256, 64, 64
HW = H * W            # 4096
P_CORE = 1024         # p rows per core
ROWS = 16             # image rows per core
N_CORES = 8
BIG = float(2 ** 30)
NEG_INF = -1e8
A_EPS = math.sqrt(1.002)

_NC = None


def _emit(nc):
    # ---- DRAM I/O ----
    feats2_d = nc.dram_tensor("feats2", [128, 2, HW], F32, kind="ExternalInput").ap()
    fhalo_d = nc.dram_tensor("fhalo", [128, 2, 20, 66], F32, kind="ExternalInput").ap()
    zrow_d = nc.dram_tensor("zrow", [128, 66], F32, kind="ExternalInput").ap()
    wp0_d = nc.dram_tensor("wp0", [128, 9, 2, 2, 128], F32, kind="ExternalInput").ap()
    wp1_d = nc.dram_tensor("wp1", [128, 9, 2, 2, 128], F32, kind="ExternalInput").ap()
    wk_d = nc.dram_tensor("wk", [128, 2, 257], F32, kind="ExternalInput").ap()
    wf_d = nc.dram_tensor("wf", [128, 2, 256], F32, kind="ExternalInput").ap()
    biases_d = nc.dram_tensor("biases", [128, 11], F32, kind="ExternalInput").ap()
    lhs_ex_d = nc.dram_tensor("lhs_ex", [17, P_CORE], BF16, kind="ExternalInput").ap()
    rhs_ex_d = nc.dram_tensor("rhs_ex", [19, HW], BF16, kind="ExternalInput").ap()

    out_d = nc.dram_tensor("out", [P_CORE, HW], F32, kind="ExternalOutput").ap()
    s_out_d = nc.dram_tensor("s_out", [128, 32], F32, kind="ExternalOutput").ap()

    with tile.TileContext(nc) as tc:
        with tc.tile_pool(name="consts", bufs=1) as consts:
            wk_sb = consts.tile([128, 2, 257], F32R, tag="wk")
            wf_sb = consts.tile([128, 2, 256], F32R, tag="wf")
            biases = consts.tile([128, 11], F32, tag="biases")
            rhs_sb = consts.tile([49, HW], BF16, tag="rhs_sb")
            extra_sb = consts.tile([49, P_CORE], BF16, tag="extra_sb")
            kern = consts.tile([128, 2, P_CORE], F32R, tag="kern")
            f_sb = consts.tile([128, 2, HW], F32R, tag="f_sb")
            h1 = consts.tile([128, 2, ROWS * 64], F32R, tag="h1")
            s1cols = consts.tile([128, 32], F32, tag="s1cols")

            nc.sync.dma_start(out=biases[:], in_=biases_d)
            nc.sync.dma_start(out=rhs_sb[0:2, :], in_=rhs_ex_d[0:2, :])
            nc.sync.dma_start(out=rhs_sb[32:49, :], in_=rhs_ex_d[2:19, :])
            nc.sync.dma_start(out=extra_sb[32:49, :], in_=lhs_ex_d)
            nc.sync.dma_start(out=extra_sb[1:2, :], in_=lhs_ex_d[0:1, :])

            # ---------- conv phase (scoped pools; freed before loss phase) ----
            with tc.tile_pool(name="convp", bufs=1) as convp, \
                 tc.tile_pool(name="fblk", bufs=3) as fblkp, \
                 tc.tile_pool(name="cpsum", bufs=2, space="PSUM") as cps, \
                 tc.tile_pool(name="fpsum", bufs=2, space="PSUM") as fps:
                wp0 = convp.tile([128, 9, 2, 2, 128], F32R, tag="wp0")
                wp1 = convp.tile([128, 9, 2, 2, 128], F32R, tag="wp1")
                fpad = convp.tile([128, 2, 20, 66], F32R, tag="fpad")
                h0p = convp.tile([128, 2, 18, 66], F32R, tag="h0p")

                nc.gpsimd.dma_start(out=wp0[:], in_=wp0_d)
                nc.gpsimd.dma_start(out=fpad[:], in_=fhalo_d)
                nc.gpsimd.dma_start(out=wp1[:], in_=wp1_d)
                nc.gpsimd.dma_start(out=wk_sb[:], in_=wk_d)
                nc.gpsimd.dma_start(out=wf_sb[:], in_=wf_d)
                nc.gpsimd.dma_start(out=h0p[:, :, :, 0:1], in_=zrow_d[:, 0:36])
                nc.gpsimd.dma_start(out=h0p[:, :, :, 65:66], in_=zrow_d[:, 0:36])

                # conv1: fpad rows (image r0-2..r0+17) -> h0p rows (r0-1..r0+16)
                for ot in range(2):
                    for (R, nb) in ((0, 8), (8, 8), (16, 2)):
                        ps = cps.tile([128, 512], F32, tag="conv")
                        n = nb * 64
                        for kt in range(2):
                            for tap in range(9):
                                dy, dx = tap // 3 - 1, tap % 3 - 1
                                nc.tensor.matmul(
                                    ps[:, :n],
                                    wp0[:, tap, kt, ot, :],
                                    fpad[:, kt, R + dy + 1:R + dy + 1 + nb,
                                         dx + 1:dx + 1 + 64],
                                    start=(kt == 0 and tap == 0),
                                    stop=(kt == 1 and tap == 8))
                        nc.vector.tensor_scalar(
                            out=h0p[:, ot, R:R + nb, 1:65], in0=ps[:, :n],
                            scalar1=biases[:, ot:ot + 1], scalar2=0.0,
                            op0=ALU.add, op1=ALU.max)

                # zero out-of-image h0 pad rows (row-kill: 0.0 on boundary cores)
                for kt in range(2):
                    nc.vector.tensor_scalar(
                        out=h0p[:, kt, 0, :], in0=h0p[:, kt, 0, :],
                        scalar1=biases[:, 9:10], scalar2=None, op0=ALU.mult)
                    nc.vector.tensor_scalar(
                        out=h0p[:, kt, 17, :], in0=h0p[:, kt, 17, :],
                        scalar1=biases[:, 10:11], scalar2=None, op0=ALU.mult)

                # conv2: h0p rows (r0-1..r0+16) -> h1 rows (r0..r0+15)
                for ot in range(2):
                    for R in (0, 8):
                        ps = cps.tile([128, 512], F32, tag="conv")
                        for kt in range(2):
                            for tap in range(9):
                                dy, dx = tap // 3 - 1, tap % 3 - 1
                                nc.tensor.matmul(
                                    ps[:],
                                    wp1[:, tap, kt, ot, :],
                                    h0p[:, kt, R + dy + 1:R + dy + 1 + 8,
                                        dx + 1:dx + 1 + 64],
                                    start=(kt == 0 and tap == 0),
                                    stop=(kt == 1 and tap == 8))
                        nc.vector.tensor_scalar(
                            out=h1[:, ot, R * 64:R * 64 + 512], in0=ps[:],
                            scalar1=biases[:, 2 + ot:3 + ot], scalar2=0.0,
                            op0=ALU.add, op1=ALU.max)

                # kernels 1x1: h1 -> kern [c, p] and kb row -> extra_sb[0]
                for ot in range(2):
                    for pb in range(2):
                        ps = cps.tile([128, 512], F32, tag="conv")
                        for kt in range(2):
                            nc.tensor.matmul(
                                ps[:], wk_sb[:, kt, ot * 128:(ot + 1) * 128],
                                h1[:, kt, pb * 512:(pb + 1) * 512],
                                start=(kt == 0), stop=(kt == 1))
                        nc.vector.tensor_scalar(
                            out=kern[:, ot, pb * 512:(pb + 1) * 512], in0=ps[:],
                            scalar1=biases[:, 4 + ot:5 + ot], scalar2=None,
                            op0=ALU.add)
                for pb in range(2):
                    ps = cps.tile([1, 512], F32, tag="kb")
                    for kt in range(2):
                        nc.tensor.matmul(
                            ps[:], wk_sb[:, kt, 256:257],
                            h1[:, kt, pb * 512:(pb + 1) * 512],
                            start=(kt == 0), stop=(kt == 1))
                    nc.scalar.activation(
                        extra_sb[0:1, pb * 512:(pb + 1) * 512], ps[:],
                        AF.Identity, bias=biases[0:1, 8:9])

                # f 1x1: feats (streamed) -> f_sb [c, hw]
                for fb in range(4):
                    fblk = fblkp.tile([128, 2, 1024], F32R, tag="fblk")
                    nc.gpsimd.dma_start(
                        out=fblk[:], in_=feats2_d[:, :, fb * 1024:(fb + 1) * 1024])
                    for ot in range(2):
                        ps = fps.tile([128, 1024], F32, tag="fps")
                        for half in range(2):
                            for kt in range(2):
                                nc.tensor.matmul(
                                    ps[:, half * 512:(half + 1) * 512],
                                    wf_sb[:, kt, ot * 128:(ot + 1) * 128],
                                    fblk[:, kt, half * 512:(half + 1) * 512],
                                    start=(kt == 0), stop=(kt == 1))
                        nc.vector.tensor_scalar(
                            out=f_sb[:, ot, fb * 1024:(fb + 1) * 1024], in0=ps[:],
                            scalar1=biases[:, 6 + ot:7 + ot], scalar2=None,
                            op0=ALU.add)

            # ---------- big loop: logits + loss ----------
            with tc.tile_pool(name="bpsum", bufs=4, space="PSUM") as bps, \
                 tc.tile_pool(name="upool", bufs=2) as upool, \
                 tc.tile_pool(name="atpool", bufs=8) as atpool, \
                 tc.tile_pool(name="lrow", bufs=3) as lrow, \
                 tc.tile_pool(name="soutp", bufs=2) as soutp:
                at_tiles = []
                for pt in range(8):
                    at_t = atpool.tile([128, HW], BF16, tag="at")
                    at_tiles.append(at_t)
                    ps_list = []
                    # phase A: dense PE work + output evict
                    for jb in range(4):
                        ps = bps.tile([128, 1024], F32, tag="big")
                        ps_list.append(ps)
                        for half in range(2):
                            sl = slice(half * 512, (half + 1) * 512)
                            col = slice(jb * 1024 + half * 512,
                                        jb * 1024 + half * 512 + 512)
                            for kt in range(2):
                                nc.tensor.matmul(
                                    ps[:, sl],
                                    kern[:, kt, pt * 128:(pt + 1) * 128],
                                    f_sb[:, kt, col],
                                    start=(kt == 0), stop=False)
                            # kb + Mneg (rows 0:2 of extras)
                            nc.tensor.matmul(
                                ps[:, sl],
                                extra_sb[0:2, pt * 128:(pt + 1) * 128],
                                rhs_sb[0:2, col],
                                start=False, stop=True)
                        # output evict: max(logits + kb + Mneg, -1e8)
                        lr = lrow.tile([128, 1024], F32, tag="lr")
                        nc.vector.tensor_scalar(
                            out=lr[:], in0=ps[:], scalar1=NEG_INF, scalar2=None,
                            op0=ALU.max)
                        nc.sync.dma_start(
                            out=out_d[pt * 128:(pt + 1) * 128,
                                      jb * 1024:(jb + 1) * 1024],
                            in_=lr[:])
                    # phase B: label-mask extras + loss activations
                    for jb in range(4):
                        ps = ps_list[jb]
                        for half in range(2):
                            sl = slice(half * 512, (half + 1) * 512)
                            col = slice(jb * 1024 + half * 512,
                                        jb * 1024 + half * 512 + 512)
                            nc.tensor.matmul(
                                ps[:, sl],
                                extra_sb[32:49, pt * 128:(pt + 1) * 128],
                                rhs_sb[32:49, col],
                                start=False, stop=True, skip_group_check=True)
                        u_t = upool.tile([128, 1024], F32, tag="u")
                        nc.scalar.activation(u_t[:], ps[:], AF.Sigmoid)
                        nc.scalar.activation(
                            at_t[:, jb * 1024:(jb + 1) * 1024], u_t[:],
                            AF.Arctan, scale=1.0 / A_EPS)

                # sin phase (single ACT table switch)
                for pt in range(8):
                    for jb in range(4):
                        so = soutp.tile([128, 1024], BF16, tag="so")
                        nc.scalar.activation(
                            so[:], at_tiles[pt][:, jb * 1024:(jb + 1) * 1024],
                            AF.Sin, scale=2.0,
                            accum_out=s1cols[:, pt * 4 + jb:pt * 4 + jb + 1])
                nc.sync.dma_start(out=s_out_d, in_=s1cols[:])


def _build():
    global _NC
    if _NC is not None:
        return _NC
    nc = bacc.Bacc("TRN2", target_bir_lowering=False, debug=False,
                   num_devices=N_CORES)
    _emit(nc)
    nc.compile()
    _NC = nc
    return nc


def _prep_core(inputs, c):
    b, q = c // 4, c % 4
    r0 = q * ROWS
    p0 = q * P_CORE
    feats = np.ascontiguousarray(inputs["feats"][b], dtype=np.float32)  # [256,64,64]
    fflat = feats.reshape(C, HW)
    masks = np.asarray(inputs["masks"][b], dtype=np.float32).reshape(HW)
    layouts = np.asarray(inputs["layouts"][b]).reshape(HW)

    feats2 = fflat.reshape(2, 128, HW).transpose(1, 0, 2)  # [128,2,4096]

    halo = np.zeros((C, 20, 66), np.float32)
    lo, hi = max(r0 - 2, 0), min(r0 + 18, H)
    halo[:, lo - (r0 - 2):hi - (r0 - 2), 1:65] = feats[:, lo:hi, :]
    fhalo = halo.reshape(2, 128, 20, 66).transpose(1, 0, 2, 3)

    def prep_w33(w):  # [o, i, 3, 3] -> [128, 9, 2, 2, 128]
        t = np.asarray(w, np.float32).transpose(2, 3, 1, 0).reshape(9, 2, 128, 2, 128)
        return np.ascontiguousarray(t.transpose(2, 0, 1, 3, 4))

    wk = np.asarray(inputs["w_kernel"], np.float32).reshape(257, 256)
    wk_t = np.ascontiguousarray(wk.T.reshape(2, 128, 257).transpose(1, 0, 2))
    wf = np.asarray(inputs["w_feats"], np.float32).reshape(256, 256)
    wf_t = np.ascontiguousarray(wf.T.reshape(2, 128, 256).transpose(1, 0, 2))

    bz = np.zeros((128, 11), np.float32)
    bz[:, 0] = inputs["b_pre0"][:128]
    bz[:, 1] = inputs["b_pre0"][128:]
    bz[:, 2] = inputs["b_pre1"][:128]
    bz[:, 3] = inputs["b_pre1"][128:]
    bz[:, 4] = inputs["b_kernel"][:128]
    bz[:, 5] = inputs["b_kernel"][128:256]
    bz[:, 6] = inputs["b_feats"][:128]
    bz[:, 7] = inputs["b_feats"][128:]
    bz[:, 8] = inputs["b_kernel"][256]
    bz[:, 9] = 0.0 if r0 == 0 else 1.0
    bz[:, 10] = 0.0 if r0 + ROWS == H else 1.0

    import ml_dtypes
    oh_hw = (layouts[None, :] == np.arange(16)[:, None]).astype(np.float32)
    lhs_ex = np.ones((17, P_CORE), np.float32)
    lhs_ex[1:17] = oh_hw[:, p0:p0 + P_CORE]
    lhs_ex = lhs_ex.astype(ml_dtypes.bfloat16)
    rhs_ex = np.zeros((19, HW), np.float32)
    rhs_ex[0] = 1.0
    rhs_ex[1] = np.where(masks > 0, 0.0, -BIG)   # Mneg
    rhs_ex[2] = -BIG
    rhs_ex[3:19] = BIG * oh_hw
    rhs_ex = rhs_ex.astype(ml_dtypes.bfloat16)

    return {
        "feats2": np.ascontiguousarray(feats2),
        "fhalo": np.ascontiguousarray(fhalo),
        "wp0": prep_w33(inputs["w_pre0"]),
        "wp1": prep_w33(inputs["w_pre1"]),
        "wk": wk_t, "wf": wf_t, "biases": bz,
        "lhs_ex": lhs_ex, "rhs_ex": rhs_ex,
        "zrow": np.zeros((128, 66), np.float32),
    }


def prep_in_maps(inputs):
    inputs = {k: np.asarray(v) for k, v in inputs.items()}
    return [_prep_core(inputs, c) for c in range(N_CORES)]


def postprocess(results, inputs):
    logits = np.empty((B, HW, HW), np.float32)
    S = np.zeros(B, np.float64)
    masks = np.asarray(inputs["masks"], dtype=np.float32).reshape(B, HW)
    for c in range(N_CORES):
        b, q = c // 4, c % 4
        p0 = q * P_CORE
        logits[b, p0:p0 + P_CORE, :] = results[c]["out"]
        s = results[c]["s_out"].astype(np.float64)          # [128, 32]
        s_vec = s.reshape(128, 8, 4).sum(-1).T.reshape(P_CORE)  # p = pt*128+i
        S[b] += (masks[b, p0:p0 + P_CORE].astype(np.float64) * s_vec).sum()
    S /= A_EPS
    grid = masks.sum(1).astype(np.float64)
    loss = np.float32(((grid * grid - S) / (grid * grid + 1e-5)).mean())
    return logits.reshape(B, HW, H, W), loss


def run(inputs, trace=False):
    nc = _build()
    in_maps = prep_in_maps(inputs)
    res = run_bass_kernel_spmd(nc, in_maps, list(range(N_CORES)), trace=trace)
    logits, loss = postprocess(res.results, inputs)
    return logits, loss, res


def kernel(**inputs):
    logits, loss, _ = run(inputs)
    return logits, loss


# revision 9
# speedup vs baseline: 1.1678x; 1.0335x over previous
"""Trainium2 Bass kernel for nn_MergeHead (dynamic-conv head + dice loss).

Sharding: 8 cores = 2 batches x 4 row-blocks of 16 image rows each.
Each core computes, for its batch b and rows [r0, r0+16):
  - h0 = relu(conv3x3(feats))           (rows r0-1 .. r0+17, via halo)
  - h1 = relu(conv3x3(h0))              (rows r0 .. r0+16)
  - kernels = 1x1(h1) -> kw^T [256 x 1024], kb [1 x 1024]
  - f = 1x1(feats_full) [256 x 4096]
  - logits[p, hw] = kw^T.T @ f + kb  (p in core's 1024 rows)
  - output: max(logits + Mneg, -1e8)  (Mneg = -2^30 at masked cols)
  - loss partial: s[p] = sum_hw sin(2*atan(sigma(x_loss)/sqrt(1.002)))
    where x_loss = logits + kb + Mneg + 2^30*(eq-1)  (eq = label match via
    one-hot K=16 matmul).  Identity: 2u/(u^2+1.002) =
    (1/sqrt(1.002)) * sin(2*atan(u/sqrt(1.002))), u = sigmoid(x).
Host combines: loss = mean_b (grid^2 - S_b) / (grid^2 + 1e-5).
"""
import math

import numpy as np

import concourse.bass as bass
import concourse.bacc as bacc
import concourse.tile as tile
from concourse import mybir
from concourse.bass_utils import run_bass_kernel_spmd

dt = mybir.dt
F32 = dt.float32
F32R = dt.float32r
BF16 = dt.bfloat16
AF = mybir.ActivationFunctionType
ALU = mybir.AluOpType

B, C, H, W = 2, 256, 64, 64
HW = H * W            # 4096
P_CORE = 1024         # p rows per core
ROWS = 16             # image rows per core
N_CORES = 8
BIG = float(2 ** 30)
NEG_INF = -1e8
A_EPS = math.sqrt(1.002)

_NC = None


def _emit(nc):
    # ---- DRAM I/O ----
    feats2_d = nc.dram_tensor("feats2", [128, 2, HW], F32, kind="ExternalInput").ap()
    fhalo_d = nc.dram_tensor("fhalo", [128, 2, 20, 66], F32, kind="ExternalInput").ap()
    zrow_d = nc.dram_tensor("zrow", [128, 66], F32, kind="ExternalInput").ap()
    wp0_d = nc.dram_tensor("wp0", [128, 9, 2, 2, 128], F32, kind="ExternalInput").ap()
    wp1_d = nc.dram_tensor("wp1", [128, 9, 2, 2, 128], F32, kind="ExternalInput").ap()
    wk_d = nc.dram_tensor("wk", [128, 2, 257], F32, kind="ExternalInput").ap()
    wf_d = nc.dram_tensor("wf", [128, 2, 256], F32, kind="ExternalInput").ap()
    biases_d = nc.dram_tensor("biases", [128, 11], F32, kind="ExternalInput").ap()
    lhs_ex_d = nc.dram_tensor("lhs_ex", [17, P_CORE], BF16, kind="ExternalInput").ap()
    rhs_ex_d = nc.dram_tensor("rhs_ex", [19, HW], BF16, kind="ExternalInput").ap()

    out_d = nc.dram_tensor("out", [P_CORE, HW], F32, kind="ExternalOutput").ap()
    s_out_d = nc.dram_tensor("s_out", [128, 32], F32, kind="ExternalOutput").ap()

    with tile.TileContext(nc) as tc:
        with tc.tile_pool(name="consts", bufs=1) as consts:
            wk_sb = consts.tile([128, 2, 257], F32R, tag="wk")
            wf_sb = consts.tile([128, 2, 256], F32R, tag="wf")
            biases = consts.tile([128, 11], F32, tag="biases")
            rhs_sb = consts.tile([49, HW], BF16, tag="rhs_sb")
            extra_sb = consts.tile([49, P_CORE], BF16, tag="extra_sb")
            kern = consts.tile([128, 2, P_CORE], F32R, tag="kern")
            f_sb = consts.tile([128, 2, HW], F32R, tag="f_sb")
            h1 = consts.tile([128, 2, ROWS * 64], F32R, tag="h1")
            s1cols = consts.tile([128, 32], F32, tag="s1cols")

            nc.sync.dma_start(out=biases[:], in_=biases_d)
            nc.sync.dma_start(out=rhs_sb[0:2, :], in_=rhs_ex_d[0:2, :])
            nc.sync.dma_start(out=rhs_sb[32:49, :], in_=rhs_ex_d[2:19, :])
            nc.sync.dma_start(out=extra_sb[32:49, :], in_=lhs_ex_d)
            nc.sync.dma_start(out=extra_sb[1:2, :], in_=lhs_ex_d[0:1, :])

            psum = ctxpool = tc.tile_pool(name="psum", bufs=4, space="PSUM")
            psum = psum.__enter__()
            # ---------- conv phase (scoped SBUF pool; freed before loss) ----
            with tc.tile_pool(name="convp", bufs=1) as convp, \
                 tc.tile_pool(name="fblk", bufs=3) as fblkp:
                wp0 = convp.tile([128, 9, 2, 2, 128], F32R, tag="wp0")
                wp1 = convp.tile([128, 9, 2, 2, 128], F32R, tag="wp1")
                fpad = convp.tile([128, 2, 20, 66], F32R, tag="fpad")
                h0p = convp.tile([128, 2, 18, 66], F32R, tag="h0p")

                nc.gpsimd.dma_start(out=wp0[:], in_=wp0_d)
                nc.gpsimd.dma_start(out=fpad[:], in_=fhalo_d)
                nc.gpsimd.dma_start(out=wp1[:], in_=wp1_d)
                nc.gpsimd.dma_start(out=wk_sb[:], in_=wk_d)
                nc.gpsimd.dma_start(out=wf_sb[:], in_=wf_d)
                nc.gpsimd.dma_start(out=h0p[:, :, :, 0:1], in_=zrow_d[:, 0:36])
                nc.gpsimd.dma_start(out=h0p[:, :, :, 65:66], in_=zrow_d[:, 0:36])

                # conv1: fpad rows (image r0-2..r0+17) -> h0p rows (r0-1..r0+16)
                for ot in range(2):
                    for (R, nb) in ((0, 8), (8, 8), (16, 2)):
                        ps = psum.tile([128, 1024], F32, tag="big", name="cps")
                        n = nb * 64
                        for kt in range(2):
                            for tap in range(9):
                                dy, dx = tap // 3 - 1, tap % 3 - 1
                                nc.tensor.matmul(
                                    ps[:, :n],
                                    wp0[:, tap, kt, ot, :],
                                    fpad[:, kt, R + dy + 1:R + dy + 1 + nb,
                                         dx + 1:dx + 1 + 64],
                                    start=(kt == 0 and tap == 0),
                                    stop=(kt == 1 and tap == 8))
                        nc.vector.tensor_scalar(
                            out=h0p[:, ot, R:R + nb, 1:65], in0=ps[:, :n],
                            scalar1=biases[:, ot:ot + 1], scalar2=0.0,
                            op0=ALU.add, op1=ALU.max)

                # zero out-of-image h0 pad rows (row-kill: 0.0 on boundary cores)
                for kt in range(2):
                    nc.vector.tensor_scalar(
                        out=h0p[:, kt, 0, :], in0=h0p[:, kt, 0, :],
                        scalar1=biases[:, 9:10], scalar2=None, op0=ALU.mult)
                    nc.vector.tensor_scalar(
                        out=h0p[:, kt, 17, :], in0=h0p[:, kt, 17, :],
                        scalar1=biases[:, 10:11], scalar2=None, op0=ALU.mult)

                # conv2: h0p rows (r0-1..r0+16) -> h1 rows (r0..r0+15)
                for ot in range(2):
                    for R in (0, 8):
                        ps = psum.tile([128, 1024], F32, tag="big", name="c2ps")
                        for kt in range(2):
                            for tap in range(9):
                                dy, dx = tap // 3 - 1, tap % 3 - 1
                                nc.tensor.matmul(
                                    ps[:, :512],
                                    wp1[:, tap, kt, ot, :],
                                    h0p[:, kt, R + dy + 1:R + dy + 1 + 8,
                                        dx + 1:dx + 1 + 64],
                                    start=(kt == 0 and tap == 0),
                                    stop=(kt == 1 and tap == 8))
                        nc.vector.tensor_scalar(
                            out=h1[:, ot, R * 64:R * 64 + 512], in0=ps[:, :512],
                            scalar1=biases[:, 2 + ot:3 + ot], scalar2=0.0,
                            op0=ALU.add, op1=ALU.max)

                # kernels 1x1: h1 -> kern [c, p] and kb row -> extra_sb[0]
                for ot in range(2):
                    ps = psum.tile([128, 1024], F32, tag="big", name="knps")
                    for pb in range(2):
                        for kt in range(2):
                            nc.tensor.matmul(
                                ps[:, pb * 512:(pb + 1) * 512],
                                wk_sb[:, kt, ot * 128:(ot + 1) * 128],
                                h1[:, kt, pb * 512:(pb + 1) * 512],
                                start=(kt == 0), stop=(kt == 1))
                    nc.vector.tensor_scalar(
                        out=kern[:, ot, :], in0=ps[:],
                        scalar1=biases[:, 4 + ot:5 + ot], scalar2=None,
                        op0=ALU.add)
                ps = psum.tile([1, 1024], F32, tag="big", name="kbps")
                for pb in range(2):
                    for kt in range(2):
                        nc.tensor.matmul(
                            ps[:, pb * 512:(pb + 1) * 512],
                            wk_sb[:, kt, 256:257],
                            h1[:, kt, pb * 512:(pb + 1) * 512],
                            start=(kt == 0), stop=(kt == 1))
                nc.scalar.activation(
                    extra_sb[0:1, :], ps[:],
                    AF.Identity, bias=biases[0:1, 8:9])

                # f 1x1: feats (streamed) -> f_sb [c, hw]
                for fb in range(4):
                    fblk = fblkp.tile([128, 2, 1024], F32R, tag="fblk")
                    nc.gpsimd.dma_start(
                        out=fblk[:], in_=feats2_d[:, :, fb * 1024:(fb + 1) * 1024])
                    for ot in range(2):
                        ps = psum.tile([128, 1024], F32, tag="big", name="fps")
                        for half in range(2):
                            for kt in range(2):
                                nc.tensor.matmul(
                                    ps[:, half * 512:(half + 1) * 512],
                                    wf_sb[:, kt, ot * 128:(ot + 1) * 128],
                                    fblk[:, kt, half * 512:(half + 1) * 512],
                                    start=(kt == 0), stop=(kt == 1))
                        nc.vector.tensor_scalar(
                            out=f_sb[:, ot, fb * 1024:(fb + 1) * 1024], in0=ps[:],
                            scalar1=biases[:, 6 + ot:7 + ot], scalar2=None,
                            op0=ALU.add)

            # ---------- big loop: logits + loss ----------
            with tc.tile_pool(name="upool", bufs=2) as upool, \
                 tc.tile_pool(name="atpool", bufs=8) as atpool, \
                 tc.tile_pool(name="lrow", bufs=3) as lrow, \
                 tc.tile_pool(name="soutp", bufs=2) as soutp:
                at_tiles = []
                for pt in range(8):
                    at_t = atpool.tile([128, HW], BF16, tag="at")
                    at_tiles.append(at_t)
                    ps_list = [psum.tile([128, 1024], F32, tag="big", name="bigps")
                               for _ in range(4)]
                    # phase A: dense PE work + output evict
                    for jb in range(4):
                        for half in range(2):
                            sl = slice(half * 512, (half + 1) * 512)
                            col = slice(jb * 1024 + half * 512,
                                        jb * 1024 + half * 512 + 512)
                            for kt in range(2):
                                nc.tensor.matmul(
                                    ps_list[jb][:, sl],
                                    kern[:, kt, pt * 128:(pt + 1) * 128],
                                    f_sb[:, kt, col],
                                    start=(kt == 0), stop=False)
                            # kb + Mneg (rows 0:2 of extras)
                            nc.tensor.matmul(
                                ps_list[jb][:, sl],
                                extra_sb[0:2, pt * 128:(pt + 1) * 128],
                                rhs_sb[0:2, col],
                                start=False, stop=True)
                    for jb in range(4):
                        # output evict: max(logits + kb + Mneg, -1e8)
                        lr = lrow.tile([128, 1024], F32, tag="lr")
                        nc.vector.tensor_scalar(
                            out=lr[:], in0=ps_list[jb][:], scalar1=NEG_INF,
                            scalar2=None, op0=ALU.max)
                        nc.sync.dma_start(
                            out=out_d[pt * 128:(pt + 1) * 128,
                                      jb * 1024:(jb + 1) * 1024],
                            in_=lr[:])
                    # phase B: label-mask extras + loss activations
                    u_t = upool.tile([128, HW], F32, tag="u")
                    for jb in range(4):
                        ps = ps_list[jb]
                        for half in range(2):
                            sl = slice(half * 512, (half + 1) * 512)
                            col = slice(jb * 1024 + half * 512,
                                        jb * 1024 + half * 512 + 512)
                            nc.tensor.matmul(
                                ps[:, sl],
                                extra_sb[32:49, pt * 128:(pt + 1) * 128],
                                rhs_sb[32:49, col],
                                start=False, stop=True, skip_group_check=True)
                        nc.scalar.activation(
                            u_t[:, jb * 1024:(jb + 1) * 1024], ps[:], AF.Sigmoid)
                    nc.scalar.activation(at_t[:], u_t[:], AF.Arctan,
                                         scale=1.0 / A_EPS)

                # sin phase (single ACT table switch)
                for pt in range(8):
                    for jb in range(4):
                        so = soutp.tile([128, 1024], BF16, tag="so")
                        nc.scalar.activation(
                            so[:], at_tiles[pt][:, jb * 1024:(jb + 1) * 1024],
                            AF.Sin, scale=2.0,
                            accum_out=s1cols[:, pt * 4 + jb:pt * 4 + jb + 1])
                nc.sync.dma_start(out=s_out_d, in_=s1cols[:])
            ctxpool.__exit__(None, None, None)


def _build():
    global _NC
    if _NC is not None:
        return _NC
    nc = bacc.Bacc("TRN2", target_bir_lowering=False, debug=False,
                   num_devices=N_CORES)
    _emit(nc)
    nc.compile()
    _NC = nc
    return nc


def _prep_core(inputs, c):
    b, q = c // 4, c % 4
    r0 = q * ROWS
    p0 = q * P_CORE
    feats = np.ascontiguousarray(inputs["feats"][b], dtype=np.float32)  # [256,64,64]
    fflat = feats.reshape(C, HW)
    masks = np.asarray(inputs["masks"][b], dtype=np.float32).reshape(HW)
    layouts = np.asarray(inputs["layouts"][b]).reshape(HW)

    feats2 = fflat.reshape(2, 128, HW).transpose(1, 0, 2)  # [128,2,4096]

    halo = np.zeros((C, 20, 66), np.float32)
    lo, hi = max(r0 - 2, 0), min(r0 + 18, H)
    halo[:, lo - (r0 - 2):hi - (r0 - 2), 1:65] = feats[:, lo:hi, :]
    fhalo = halo.reshape(2, 128, 20, 66).transpose(1, 0, 2, 3)

    def prep_w33(w):  # [o, i, 3, 3] -> [128, 9, 2, 2, 128]
        t = np.asarray(w, np.float32).transpose(2, 3, 1, 0).reshape(9, 2, 128, 2, 128)
        return np.ascontiguousarray(t.transpose(2, 0, 1, 3, 4))

    wk = np.asarray(inputs["w_kernel"], np.float32).reshape(257, 256)
    wk_t = np.ascontiguousarray(wk.T.reshape(2, 128, 257).transpose(1, 0, 2))
    wf = np.asarray(inputs["w_feats"], np.float32).reshape(256, 256)
    wf_t = np.ascontiguousarray(wf.T.reshape(2, 128, 256).transpose(1, 0, 2))

    bz = np.zeros((128, 11), np.float32)
    bz[:, 0] = inputs["b_pre0"][:128]
    bz[:, 1] = inputs["b_pre0"][128:]
    bz[:, 2] = inputs["b_pre1"][:128]
    bz[:, 3] = inputs["b_pre1"][128:]
    bz[:, 4] = inputs["b_kernel"][:128]
    bz[:, 5] = inputs["b_kernel"][128:256]
    bz[:, 6] = inputs["b_feats"][:128]
    bz[:, 7] = inputs["b_feats"][128:]
    bz[:, 8] = inputs["b_kernel"][256]
    bz[:, 9] = 0.0 if r0 == 0 else 1.0
    bz[:, 10] = 0.0 if r0 + ROWS == H else 1.0

    import ml_dtypes
    oh_hw = (layouts[None, :] == np.arange(16)[:, None]).astype(np.float32)
    lhs_ex = np.ones((17, P_CORE), np.float32)
    lhs_ex[1:17] = oh_hw[:, p0:p0 + P_CORE]
    lhs_ex = lhs_ex.astype(ml_dtypes.bfloat16)
    rhs_ex = np.zeros((19, HW), np.float32)
    rhs_ex[0] = 1.0
    rhs_ex[1] = np.where(masks > 0, 0.0, -BIG)   # Mneg
    rhs_ex[2] = -BIG
    rhs_ex[3:19] = BIG * oh_hw
    rhs_ex = rhs_ex.astype(ml_dtypes.bfloat16)

    return {
        "feats2": np.ascontiguousarray(feats2),
        "fhalo": np.ascontiguousarray(fhalo),
        "wp0": prep_w33(inputs["w_pre0"]),
        "wp1": prep_w33(inputs["w_pre1"]),
        "wk": wk_t, "wf": wf_t, "biases": bz,
        "lhs_ex": lhs_ex, "rhs_ex": rhs_ex,
        "zrow": np.zeros((128, 66), np.float32),
    }


def prep_in_maps(inputs):
    inputs = {k: np.asarray(v) for k, v in inputs.items()}
    return [_prep_core(inputs, c) for c in range(N_CORES)]


def postprocess(results, inputs):
    logits = np.empty((B, HW, HW), np.float32)
    S = np.zeros(B, np.float64)
    masks = np.asarray(inputs["masks"], dtype=np.float32).reshape(B, HW)
    for c in range(N_CORES):
        b, q = c // 4, c % 4
        p0 = q * P_CORE
        logits[b, p0:p0 + P_CORE, :] = results[c]["out"]
        s = results[c]["s_out"].astype(np.float64)          # [128, 32]
        s_vec = s.reshape(128, 8, 4).sum(-1).T.reshape(P_CORE)  # p = pt*128+i
        S[b] += (masks[b, p0:p0 + P_CORE].astype(np.float64) * s_vec).sum()
    S /= A_EPS
    grid = masks.sum(1).astype(np.float64)
    loss = np.float32(((grid * grid - S) / (grid * grid + 1e-5)).mean())
    return logits.reshape(B, HW, H, W), loss


def run(inputs, trace=False):
    nc = _build()
    in_maps = prep_in_maps(inputs)
    res = run_bass_kernel_spmd(nc, in_maps, list(range(N_CORES)), trace=trace)
    logits, loss = postprocess(res.results, inputs)
    return logits, loss, res


def kernel(**inputs):
    logits, loss, _ = run(inputs)
    return logits, loss


# revision 10
# speedup vs baseline: 1.2026x; 1.0298x over previous
"""Trainium2 Bass kernel for nn_MergeHead (dynamic-conv head + dice loss).

Sharding: 8 cores = 2 batches x 4 row-blocks of 16 image rows each.
Each core computes, for its batch b and rows [r0, r0+16):
  - h0 = relu(conv3x3(feats))           (rows r0-1 .. r0+17, via halo)
  - h1 = relu(conv3x3(h0))              (rows r0 .. r0+16)
  - kernels = 1x1(h1) -> kw^T [256 x 1024], kb [1 x 1024]
  - f = 1x1(feats_full) [256 x 4096]
  - logits[p, hw] = kw^T.T @ f + kb  (p in core's 1024 rows)
  - output: max(logits + Mneg, -1e8)  (Mneg = -2^30 at masked cols)
  - loss partial: s[p] = sum_hw sin(2*atan(sigma(x_loss)/sqrt(1.002)))
    where x_loss = logits + kb + Mneg + 2^30*(eq-1)  (eq = label match via
    one-hot K=16 matmul).  Identity: 2u/(u^2+1.002) =
    (1/sqrt(1.002)) * sin(2*atan(u/sqrt(1.002))), u = sigmoid(x).
Host combines: loss = mean_b (grid^2 - S_b) / (grid^2 + 1e-5).
"""
import math

import numpy as np

import concourse.bass as bass
import concourse.bacc as bacc
import concourse.tile as tile
from concourse import mybir
from concourse.bass_utils import run_bass_kernel_spmd

dt = mybir.dt
F32 = dt.float32
F32R = dt.float32r
BF16 = dt.bfloat16
AF = mybir.ActivationFunctionType
ALU = mybir.AluOpType

B, C, H, W = 2, 256, 64, 64
HW = H * W            # 4096
P_CORE = 1024         # p rows per core
ROWS = 16             # image rows per core
N_CORES = 8
BIG = float(2 ** 30)
NEG_INF = -1e8
A_EPS = math.sqrt(1.002)

_NC = None


def _emit(nc):
    # ---- DRAM I/O ----
    feats2_d = nc.dram_tensor("feats2", [128, 2, HW], F32, kind="ExternalInput").ap()
    fhalo_d = nc.dram_tensor("fhalo", [128, 2, 20, 66], F32, kind="ExternalInput").ap()
    zrow_d = nc.dram_tensor("zrow", [128, 66], F32, kind="ExternalInput").ap()
    wp0_d = nc.dram_tensor("wp0", [128, 9, 2, 2, 128], F32, kind="ExternalInput").ap()
    wp1_d = nc.dram_tensor("wp1", [128, 9, 2, 2, 128], F32, kind="ExternalInput").ap()
    wk_d = nc.dram_tensor("wk", [128, 2, 257], F32, kind="ExternalInput").ap()
    wf_d = nc.dram_tensor("wf", [128, 2, 256], F32, kind="ExternalInput").ap()
    biases_d = nc.dram_tensor("biases", [128, 11], F32, kind="ExternalInput").ap()
    lhs_ex_d = nc.dram_tensor("lhs_ex", [17, P_CORE], BF16, kind="ExternalInput").ap()
    rhs_ex_d = nc.dram_tensor("rhs_ex", [19, HW], BF16, kind="ExternalInput").ap()

    out_d = nc.dram_tensor("out", [P_CORE, HW], F32, kind="ExternalOutput").ap()
    s_out_d = nc.dram_tensor("s_out", [128, 32], F32, kind="ExternalOutput").ap()

    with tile.TileContext(nc) as tc:
        with tc.tile_pool(name="consts", bufs=1) as consts:
            wk_sb = consts.tile([128, 2, 257], F32R, tag="wk")
            wf_sb = consts.tile([128, 2, 256], F32R, tag="wf")
            biases = consts.tile([128, 11], F32, tag="biases")
            rhs_sb = consts.tile([49, HW], BF16, tag="rhs_sb")
            extra_sb = consts.tile([49, P_CORE], BF16, tag="extra_sb")
            kern = consts.tile([128, 2, P_CORE], F32R, tag="kern")
            f_sb = consts.tile([128, 2, HW], F32R, tag="f_sb")
            h1 = consts.tile([128, 2, ROWS * 64], F32R, tag="h1")
            s1cols = consts.tile([128, 32], F32, tag="s1cols")

            nc.sync.dma_start(out=biases[:], in_=biases_d)
            nc.sync.dma_start(out=rhs_sb[0:2, :], in_=rhs_ex_d[0:2, :])
            nc.sync.dma_start(out=rhs_sb[32:49, :], in_=rhs_ex_d[2:19, :])
            nc.sync.dma_start(out=extra_sb[32:49, :], in_=lhs_ex_d)
            nc.sync.dma_start(out=extra_sb[1:2, :], in_=lhs_ex_d[0:1, :])

            psum = ctxpool = tc.tile_pool(name="psum", bufs=4, space="PSUM")
            psum = psum.__enter__()
            # ---------- conv phase (scoped SBUF pool; freed before loss) ----
            with tc.tile_pool(name="convp", bufs=1) as convp, \
                 tc.tile_pool(name="fblk", bufs=3) as fblkp:
                wp0 = convp.tile([128, 9, 2, 2, 128], F32R, tag="wp0")
                wp1 = convp.tile([128, 9, 2, 2, 128], F32R, tag="wp1")
                fpad = convp.tile([128, 2, 20, 66], F32R, tag="fpad")
                h0p = convp.tile([128, 2, 18, 66], F32R, tag="h0p")

                nc.gpsimd.dma_start(out=wp0[:], in_=wp0_d)
                nc.gpsimd.dma_start(out=fpad[:], in_=fhalo_d)
                nc.gpsimd.dma_start(out=wp1[:], in_=wp1_d)
                nc.gpsimd.dma_start(out=wk_sb[:], in_=wk_d)
                nc.gpsimd.dma_start(out=wf_sb[:], in_=wf_d)
                nc.gpsimd.dma_start(out=h0p[:, :, :, 0:1], in_=zrow_d[:, 0:36])
                nc.gpsimd.dma_start(out=h0p[:, :, :, 65:66], in_=zrow_d[:, 0:36])

                # conv1: fpad rows (image r0-2..r0+17) -> h0p rows (r0-1..r0+16)
                for ot in range(2):
                    for (R, nb) in ((0, 8), (8, 8), (16, 2)):
                        ps = psum.tile([128, 1024], F32, tag="big", name="cps")
                        n = nb * 64
                        for kt in range(2):
                            for tap in range(9):
                                dy, dx = tap // 3 - 1, tap % 3 - 1
                                nc.tensor.matmul(
                                    ps[:, :n],
                                    wp0[:, tap, kt, ot, :],
                                    fpad[:, kt, R + dy + 1:R + dy + 1 + nb,
                                         dx + 1:dx + 1 + 64],
                                    start=(kt == 0 and tap == 0),
                                    stop=(kt == 1 and tap == 8))
                        nc.vector.tensor_scalar(
                            out=h0p[:, ot, R:R + nb, 1:65], in0=ps[:, :n],
                            scalar1=biases[:, ot:ot + 1], scalar2=0.0,
                            op0=ALU.add, op1=ALU.max)

                # zero out-of-image h0 pad rows (row-kill: 0.0 on boundary cores)
                for kt in range(2):
                    nc.vector.tensor_scalar(
                        out=h0p[:, kt, 0, :], in0=h0p[:, kt, 0, :],
                        scalar1=biases[:, 9:10], scalar2=None, op0=ALU.mult)
                    nc.vector.tensor_scalar(
                        out=h0p[:, kt, 17, :], in0=h0p[:, kt, 17, :],
                        scalar1=biases[:, 10:11], scalar2=None, op0=ALU.mult)

                # conv2: h0p rows (r0-1..r0+16) -> h1 rows (r0..r0+15)
                for ot in range(2):
                    for R in (0, 8):
                        ps = psum.tile([128, 1024], F32, tag="big", name="c2ps")
                        for kt in range(2):
                            for tap in range(9):
                                dy, dx = tap // 3 - 1, tap % 3 - 1
                                nc.tensor.matmul(
                                    ps[:, :512],
                                    wp1[:, tap, kt, ot, :],
                                    h0p[:, kt, R + dy + 1:R + dy + 1 + 8,
                                        dx + 1:dx + 1 + 64],
                                    start=(kt == 0 and tap == 0),
                                    stop=(kt == 1 and tap == 8))
                        nc.vector.tensor_scalar(
                            out=h1[:, ot, R * 64:R * 64 + 512], in0=ps[:, :512],
                            scalar1=biases[:, 2 + ot:3 + ot], scalar2=0.0,
                            op0=ALU.add, op1=ALU.max)

                # kernels 1x1: h1 -> kern [c, p] and kb row -> extra_sb[0]
                for ot in range(2):
                    ps = psum.tile([128, 1024], F32, tag="big", name="knps")
                    for pb in range(2):
                        for kt in range(2):
                            nc.tensor.matmul(
                                ps[:, pb * 512:(pb + 1) * 512],
                                wk_sb[:, kt, ot * 128:(ot + 1) * 128],
                                h1[:, kt, pb * 512:(pb + 1) * 512],
                                start=(kt == 0), stop=(kt == 1))
                    nc.vector.tensor_scalar(
                        out=kern[:, ot, :], in0=ps[:],
                        scalar1=biases[:, 4 + ot:5 + ot], scalar2=None,
                        op0=ALU.add)
                ps = psum.tile([1, 1024], F32, tag="big", name="kbps")
                for pb in range(2):
                    for kt in range(2):
                        nc.tensor.matmul(
                            ps[:, pb * 512:(pb + 1) * 512],
                            wk_sb[:, kt, 256:257],
                            h1[:, kt, pb * 512:(pb + 1) * 512],
                            start=(kt == 0), stop=(kt == 1))
                nc.scalar.activation(
                    extra_sb[0:1, :], ps[:],
                    AF.Identity, bias=biases[0:1, 8:9])

                # f 1x1: feats (streamed) -> f_sb [c, hw]
                for fb in range(4):
                    fblk = fblkp.tile([128, 2, 1024], F32R, tag="fblk")
                    nc.gpsimd.dma_start(
                        out=fblk[:], in_=feats2_d[:, :, fb * 1024:(fb + 1) * 1024])
                    for ot in range(2):
                        ps = psum.tile([128, 1024], F32, tag="big", name="fps")
                        for half in range(2):
                            for kt in range(2):
                                nc.tensor.matmul(
                                    ps[:, half * 512:(half + 1) * 512],
                                    wf_sb[:, kt, ot * 128:(ot + 1) * 128],
                                    fblk[:, kt, half * 512:(half + 1) * 512],
                                    start=(kt == 0), stop=(kt == 1))
                        nc.vector.tensor_scalar(
                            out=f_sb[:, ot, fb * 1024:(fb + 1) * 1024], in0=ps[:],
                            scalar1=biases[:, 6 + ot:7 + ot], scalar2=None,
                            op0=ALU.add)

            # ---------- big loop: logits + loss ----------
            with tc.tile_pool(name="upool", bufs=2) as upool, \
                 tc.tile_pool(name="atpool", bufs=8) as atpool, \
                 tc.tile_pool(name="lrow", bufs=3) as lrow, \
                 tc.tile_pool(name="soutp", bufs=2) as soutp:
                at_tiles = []
                for pt in range(8):
                    at_t = atpool.tile([128, HW], BF16, tag="at")
                    at_tiles.append(at_t)
                    ps_list = [psum.tile([128, 1024], F32, tag="big", name="bigps")
                               for _ in range(4)]
                    # phase A: dense PE work + output evict
                    for jb in range(4):
                        for half in range(2):
                            sl = slice(half * 512, (half + 1) * 512)
                            col = slice(jb * 1024 + half * 512,
                                        jb * 1024 + half * 512 + 512)
                            for kt in range(2):
                                nc.tensor.matmul(
                                    ps_list[jb][:, sl],
                                    kern[:, kt, pt * 128:(pt + 1) * 128],
                                    f_sb[:, kt, col],
                                    start=(kt == 0), stop=False)
                            # kb + Mneg (rows 0:2 of extras)
                            nc.tensor.matmul(
                                ps_list[jb][:, sl],
                                extra_sb[0:2, pt * 128:(pt + 1) * 128],
                                rhs_sb[0:2, col],
                                start=False, stop=True)
                    for jb in range(4):
                        # output evict: max(logits + kb + Mneg, -1e8)
                        lr = lrow.tile([128, 1024], F32, tag="lr")
                        nc.vector.tensor_scalar(
                            out=lr[:], in0=ps_list[jb][:], scalar1=NEG_INF,
                            scalar2=None, op0=ALU.max)
                        nc.sync.dma_start(
                            out=out_d[pt * 128:(pt + 1) * 128,
                                      jb * 1024:(jb + 1) * 1024],
                            in_=lr[:])
                    # phase B: label-mask extras + loss activations
                    u_t = upool.tile([128, HW], F32, tag="u")
                    for jb in range(4):
                        ps = ps_list[jb]
                        for half in range(2):
                            sl = slice(half * 512, (half + 1) * 512)
                            col = slice(jb * 1024 + half * 512,
                                        jb * 1024 + half * 512 + 512)
                            nc.tensor.matmul(
                                ps[:, sl],
                                extra_sb[32:49, pt * 128:(pt + 1) * 128],
                                rhs_sb[32:49, col],
                                start=False, stop=True, skip_group_check=True)
                        nc.scalar.activation(
                            u_t[:, jb * 1024:(jb + 1) * 1024], ps[:], AF.Sigmoid)
                    nc.scalar.activation(at_t[:], u_t[:], AF.Arctan,
                                         scale=1.0 / A_EPS)
                    if pt in (3, 7):
                        # sin batch (one table switch per batch); sins for
                        # pts 0-3 overlap the PE work of pts 4-7
                        for spt in range(pt - 3, pt + 1):
                            for jb in range(4):
                                so = soutp.tile([128, 1024], BF16, tag="so")
                                nc.scalar.activation(
                                    so[:],
                                    at_tiles[spt][:, jb * 1024:(jb + 1) * 1024],
                                    AF.Sin, scale=2.0,
                                    accum_out=s1cols[:, spt * 4 + jb:
                                                     spt * 4 + jb + 1])

                nc.sync.dma_start(out=s_out_d, in_=s1cols[:])
            ctxpool.__exit__(None, None, None)


def _build():
    global _NC
    if _NC is not None:
        return _NC
    nc = bacc.Bacc("TRN2", target_bir_lowering=False, debug=False,
                   num_devices=N_CORES)
    _emit(nc)
    nc.compile()
    _NC = nc
    return nc


def _prep_core(inputs, c):
    b, q = c // 4, c % 4
    r0 = q * ROWS
    p0 = q * P_CORE
    feats = np.ascontiguousarray(inputs["feats"][b], dtype=np.float32)  # [256,64,64]
    fflat = feats.reshape(C, HW)
    masks = np.asarray(inputs["masks"][b], dtype=np.float32).reshape(HW)
    layouts = np.asarray(inputs["layouts"][b]).reshape(HW)

    feats2 = fflat.reshape(2, 128, HW).transpose(1, 0, 2)  # [128,2,4096]

    halo = np.zeros((C, 20, 66), np.float32)
    lo, hi = max(r0 - 2, 0), min(r0 + 18, H)
    halo[:, lo - (r0 - 2):hi - (r0 - 2), 1:65] = feats[:, lo:hi, :]
    fhalo = halo.reshape(2, 128, 20, 66).transpose(1, 0, 2, 3)

    def prep_w33(w):  # [o, i, 3, 3] -> [128, 9, 2, 2, 128]
        t = np.asarray(w, np.float32).transpose(2, 3, 1, 0).reshape(9, 2, 128, 2, 128)
        return np.ascontiguousarray(t.transpose(2, 0, 1, 3, 4))

    wk = np.asarray(inputs["w_kernel"], np.float32).reshape(257, 256)
    wk_t = np.ascontiguousarray(wk.T.reshape(2, 128, 257).transpose(1, 0, 2))
    wf = np.asarray(inputs["w_feats"], np.float32).reshape(256, 256)
    wf_t = np.ascontiguousarray(wf.T.reshape(2, 128, 256).transpose(1, 0, 2))

    bz = np.zeros((128, 11), np.float32)
    bz[:, 0] = inputs["b_pre0"][:128]
    bz[:, 1] = inputs["b_pre0"][128:]
    bz[:, 2] = inputs["b_pre1"][:128]
    bz[:, 3] = inputs["b_pre1"][128:]
    bz[:, 4] = inputs["b_kernel"][:128]
    bz[:, 5] = inputs["b_kernel"][128:256]
    bz[:, 6] = inputs["b_feats"][:128]
    bz[:, 7] = inputs["b_feats"][128:]
    bz[:, 8] = inputs["b_kernel"][256]
    bz[:, 9] = 0.0 if r0 == 0 else 1.0
    bz[:, 10] = 0.0 if r0 + ROWS == H else 1.0

    import ml_dtypes
    oh_hw = (layouts[None, :] == np.arange(16)[:, None]).astype(np.float32)
    lhs_ex = np.ones((17, P_CORE), np.float32)
    lhs_ex[1:17] = oh_hw[:, p0:p0 + P_CORE]
    lhs_ex = lhs_ex.astype(ml_dtypes.bfloat16)
    rhs_ex = np.zeros((19, HW), np.float32)
    rhs_ex[0] = 1.0
    rhs_ex[1] = np.where(masks > 0, 0.0, -BIG)   # Mneg
    rhs_ex[2] = -BIG
    rhs_ex[3:19] = BIG * oh_hw
    rhs_ex = rhs_ex.astype(ml_dtypes.bfloat16)

    return {
        "feats2": np.ascontiguousarray(feats2),
        "fhalo": np.ascontiguousarray(fhalo),
        "wp0": prep_w33(inputs["w_pre0"]),
        "wp1": prep_w33(inputs["w_pre1"]),
        "wk": wk_t, "wf": wf_t, "biases": bz,
        "lhs_ex": lhs_ex, "rhs_ex": rhs_ex,
        "zrow": np.zeros((128, 66), np.float32),
    }


def prep_in_maps(inputs):
    inputs = {k: np.asarray(v) for k, v in inputs.items()}
    return [_prep_core(inputs, c) for c in range(N_CORES)]


def postprocess(results, inputs):
    logits = np.empty((B, HW, HW), np.float32)
    S = np.zeros(B, np.float64)
    masks = np.asarray(inputs["masks"], dtype=np.float32).reshape(B, HW)
    for c in range(N_CORES):
        b, q = c // 4, c % 4
        p0 = q * P_CORE
        logits[b, p0:p0 + P_CORE, :] = results[c]["out"]
        s = results[c]["s_out"].astype(np.float64)          # [128, 32]
        s_vec = s.reshape(128, 8, 4).sum(-1).T.reshape(P_CORE)  # p = pt*128+i
        S[b] += (masks[b, p0:p0 + P_CORE].astype(np.float64) * s_vec).sum()
    S /= A_EPS
    grid = masks.sum(1).astype(np.float64)
    loss = np.float32(((grid * grid - S) / (grid * grid + 1e-5)).mean())
    return logits.reshape(B, HW, H, W), loss


def run(inputs, trace=False):
    nc = _build()
    in_maps = prep_in_maps(inputs)
    res = run_bass_kernel_spmd(nc, in_maps, list(range(N_CORES)), trace=trace)
    logits, loss = postprocess(res.results, inputs)
    return logits, loss, res


def kernel(**inputs):
    logits, loss, _ = run(inputs)
    return logits, loss


# revision 11
# speedup vs baseline: 1.2646x; 1.0515x over previous
"""Trainium2 Bass kernel for nn_MergeHead (dynamic-conv head + dice loss).

Sharding: 8 cores = 2 batches x 4 row-blocks of 16 image rows each.
Each core computes, for its batch b and rows [r0, r0+16):
  - h0 = relu(conv3x3(feats))           (rows r0-1 .. r0+17, via halo)
  - h1 = relu(conv3x3(h0))              (rows r0 .. r0+16)
  - kernels = 1x1(h1) -> kw^T [256 x 1024], kb [1 x 1024]
  - f = 1x1(feats_full) [256 x 4096]
  - logits[p, hw] = kw^T.T @ f + kb  (p in core's 1024 rows)
  - output: max(logits + Mneg, -1e8)  (Mneg = -2^30 at masked cols)
  - loss partial: s[p] = sum_hw sin(2*atan(sigma(x_loss)/sqrt(1.002)))
    where x_loss = logits + kb + Mneg + 2^30*(eq-1)  (eq = label match via
    one-hot K=16 matmul).  Identity: 2u/(u^2+1.002) =
    (1/sqrt(1.002)) * sin(2*atan(u/sqrt(1.002))), u = sigmoid(x).
Host combines: loss = mean_b (grid^2 - S_b) / (grid^2 + 1e-5).
"""
import math

import numpy as np

import concourse.bass as bass
import concourse.bacc as bacc
import concourse.tile as tile
from concourse import mybir
from concourse.bass_utils import run_bass_kernel_spmd

dt = mybir.dt
F32 = dt.float32
F32R = dt.float32r
BF16 = dt.bfloat16
AF = mybir.ActivationFunctionType
ALU = mybir.AluOpType

B, C, H, W = 2, 256, 64, 64
HW = H * W            # 4096
P_CORE = 1024         # p rows per core
ROWS = 16             # image rows per core
N_CORES = 8
BIG = float(2 ** 30)
NEG_INF = -1e8
A_EPS = math.sqrt(1.002)

_NC = None


def _emit(nc):
    # ---- DRAM I/O ----
    feats2_d = nc.dram_tensor("feats2", [128, 2, HW], F32, kind="ExternalInput").ap()
    fhalo_d = nc.dram_tensor("fhalo", [128, 2, 20, 66], F32, kind="ExternalInput").ap()
    zrow_d = nc.dram_tensor("zrow", [128, 66], F32, kind="ExternalInput").ap()
    wp0_d = nc.dram_tensor("wp0", [128, 9, 2, 2, 128], F32, kind="ExternalInput").ap()
    wp1_d = nc.dram_tensor("wp1", [128, 9, 2, 2, 128], F32, kind="ExternalInput").ap()
    wk_d = nc.dram_tensor("wk", [128, 2, 257], F32, kind="ExternalInput").ap()
    wf_d = nc.dram_tensor("wf", [128, 2, 256], F32, kind="ExternalInput").ap()
    biases_d = nc.dram_tensor("biases", [128, 11], F32, kind="ExternalInput").ap()
    lhs_ex_d = nc.dram_tensor("lhs_ex", [17, P_CORE], BF16, kind="ExternalInput").ap()
    rhs_ex_d = nc.dram_tensor("rhs_ex", [19, HW], BF16, kind="ExternalInput").ap()

    out_d = nc.dram_tensor("out", [P_CORE, HW], F32, kind="ExternalOutput").ap()
    s_out_d = nc.dram_tensor("s_out", [128, 32], F32, kind="ExternalOutput").ap()

    with tile.TileContext(nc) as tc:
        with tc.tile_pool(name="consts", bufs=1) as consts:
            wk_sb = consts.tile([128, 2, 257], F32R, tag="wk")
            wf_sb = consts.tile([128, 2, 256], F32R, tag="wf")
            biases = consts.tile([128, 11], F32, tag="biases")
            rhs_sb = consts.tile([49, HW], BF16, tag="rhs_sb")
            extra_sb = consts.tile([49, P_CORE], BF16, tag="extra_sb")
            kern = consts.tile([128, 2, P_CORE], F32R, tag="kern")
            f_sb = consts.tile([128, 2, HW], F32R, tag="f_sb")
            h1 = consts.tile([128, 2, ROWS * 64], F32R, tag="h1")
            s1cols = consts.tile([128, 32], F32, tag="s1cols")

            nc.sync.dma_start(out=biases[:], in_=biases_d)
            nc.sync.dma_start(out=rhs_sb[0:2, :], in_=rhs_ex_d[0:2, :])
            nc.sync.dma_start(out=rhs_sb[32:49, :], in_=rhs_ex_d[2:19, :])
            nc.sync.dma_start(out=extra_sb[32:49, :], in_=lhs_ex_d)
            nc.sync.dma_start(out=extra_sb[1:2, :], in_=lhs_ex_d[0:1, :])

            psum = ctxpool = tc.tile_pool(name="psum", bufs=4, space="PSUM")
            psum = psum.__enter__()
            # ---------- conv phase (scoped SBUF pool; freed before loss) ----
            with tc.tile_pool(name="convp", bufs=1) as convp, \
                 tc.tile_pool(name="fblk", bufs=3) as fblkp:
                wp0 = convp.tile([128, 9, 2, 2, 128], F32R, tag="wp0")
                wp1 = convp.tile([128, 9, 2, 2, 128], F32R, tag="wp1")
                fpad = convp.tile([128, 2, 20, 66], F32R, tag="fpad")
                h0p = convp.tile([128, 2, 18, 66], F32R, tag="h0p")

                nc.gpsimd.dma_start(out=wp0[:], in_=wp0_d)
                nc.gpsimd.dma_start(out=fpad[:], in_=fhalo_d)
                nc.gpsimd.dma_start(out=wp1[:], in_=wp1_d)
                nc.gpsimd.dma_start(out=wk_sb[:], in_=wk_d)
                nc.gpsimd.dma_start(out=wf_sb[:], in_=wf_d)
                nc.gpsimd.dma_start(out=h0p[:, :, :, 0:1], in_=zrow_d[:, 0:36])
                nc.gpsimd.dma_start(out=h0p[:, :, :, 65:66], in_=zrow_d[:, 0:36])

                # conv1: fpad rows (image r0-2..r0+17) -> h0p rows (r0-1..r0+16)
                for ot in range(2):
                    for (R, nb) in ((0, 8), (8, 8), (16, 2)):
                        ps = psum.tile([128, 1024], F32, tag="big", name="cps")
                        n = nb * 64
                        for kt in range(2):
                            for tap in range(9):
                                dy, dx = tap // 3 - 1, tap % 3 - 1
                                nc.tensor.matmul(
                                    ps[:, :n],
                                    wp0[:, tap, kt, ot, :],
                                    fpad[:, kt, R + dy + 1:R + dy + 1 + nb,
                                         dx + 1:dx + 1 + 64],
                                    start=(kt == 0 and tap == 0),
                                    stop=(kt == 1 and tap == 8))
                        nc.vector.tensor_scalar(
                            out=h0p[:, ot, R:R + nb, 1:65], in0=ps[:, :n],
                            scalar1=biases[:, ot:ot + 1], scalar2=0.0,
                            op0=ALU.add, op1=ALU.max)

                # zero out-of-image h0 pad rows (row-kill: 0.0 on boundary cores)
                for kt in range(2):
                    nc.vector.tensor_scalar(
                        out=h0p[:, kt, 0, :], in0=h0p[:, kt, 0, :],
                        scalar1=biases[:, 9:10], scalar2=None, op0=ALU.mult)
                    nc.vector.tensor_scalar(
                        out=h0p[:, kt, 17, :], in0=h0p[:, kt, 17, :],
                        scalar1=biases[:, 10:11], scalar2=None, op0=ALU.mult)

                # conv2: h0p rows (r0-1..r0+16) -> h1 rows (r0..r0+15)
                for ot in range(2):
                    for R in (0, 8):
                        ps = psum.tile([128, 1024], F32, tag="big", name="c2ps")
                        for kt in range(2):
                            for tap in range(9):
                                dy, dx = tap // 3 - 1, tap % 3 - 1
                                nc.tensor.matmul(
                                    ps[:, :512],
                                    wp1[:, tap, kt, ot, :],
                                    h0p[:, kt, R + dy + 1:R + dy + 1 + 8,
                                        dx + 1:dx + 1 + 64],
                                    start=(kt == 0 and tap == 0),
                                    stop=(kt == 1 and tap == 8))
                        nc.vector.tensor_scalar(
                            out=h1[:, ot, R * 64:R * 64 + 512], in0=ps[:, :512],
                            scalar1=biases[:, 2 + ot:3 + ot], scalar2=0.0,
                            op0=ALU.add, op1=ALU.max)

                # kernels 1x1: h1 -> kern [c, p] and kb row -> extra_sb[0]
                for ot in range(2):
                    ps = psum.tile([128, 1024], F32, tag="big", name="knps")
                    for pb in range(2):
                        for kt in range(2):
                            nc.tensor.matmul(
                                ps[:, pb * 512:(pb + 1) * 512],
                                wk_sb[:, kt, ot * 128:(ot + 1) * 128],
                                h1[:, kt, pb * 512:(pb + 1) * 512],
                                start=(kt == 0), stop=(kt == 1))
                    nc.vector.tensor_scalar(
                        out=kern[:, ot, :], in0=ps[:],
                        scalar1=biases[:, 4 + ot:5 + ot], scalar2=None,
                        op0=ALU.add)
                ps = psum.tile([1, 1024], F32, tag="big", name="kbps")
                for pb in range(2):
                    for kt in range(2):
                        nc.tensor.matmul(
                            ps[:, pb * 512:(pb + 1) * 512],
                            wk_sb[:, kt, 256:257],
                            h1[:, kt, pb * 512:(pb + 1) * 512],
                            start=(kt == 0), stop=(kt == 1))
                nc.scalar.activation(
                    extra_sb[0:1, :], ps[:],
                    AF.Identity, bias=biases[0:1, 8:9])

                # f 1x1: feats (streamed) -> f_sb [c, hw]
                for fb in range(4):
                    fblk = fblkp.tile([128, 2, 1024], F32R, tag="fblk")
                    nc.gpsimd.dma_start(
                        out=fblk[:], in_=feats2_d[:, :, fb * 1024:(fb + 1) * 1024])
                    for ot in range(2):
                        ps = psum.tile([128, 1024], F32, tag="big", name="fps")
                        for half in range(2):
                            for kt in range(2):
                                nc.tensor.matmul(
                                    ps[:, half * 512:(half + 1) * 512],
                                    wf_sb[:, kt, ot * 128:(ot + 1) * 128],
                                    fblk[:, kt, half * 512:(half + 1) * 512],
                                    start=(kt == 0), stop=(kt == 1))
                        nc.vector.tensor_scalar(
                            out=f_sb[:, ot, fb * 1024:(fb + 1) * 1024], in0=ps[:],
                            scalar1=biases[:, 6 + ot:7 + ot], scalar2=None,
                            op0=ALU.add)

            # ---------- big loop: logits + loss ----------
            with tc.tile_pool(name="upool", bufs=2) as upool, \
                 tc.tile_pool(name="atpool", bufs=8) as atpool, \
                 tc.tile_pool(name="lrow", bufs=3) as lrow, \
                 tc.tile_pool(name="soutp", bufs=2) as soutp:
                at_tiles = []
                for pt in range(8):
                    at_t = atpool.tile([128, HW], BF16, tag="at")
                    at_tiles.append(at_t)
                    ps_list = [psum.tile([128, 1024], F32, tag="big", name="bigps")
                               for _ in range(4)]
                    # phase A: dense PE work + output evict
                    for jb in range(4):
                        for half in range(2):
                            sl = slice(half * 512, (half + 1) * 512)
                            col = slice(jb * 1024 + half * 512,
                                        jb * 1024 + half * 512 + 512)
                            for kt in range(2):
                                nc.tensor.matmul(
                                    ps_list[jb][:, sl],
                                    kern[:, kt, pt * 128:(pt + 1) * 128],
                                    f_sb[:, kt, col],
                                    start=(kt == 0), stop=False)
                            # kb + Mneg (rows 0:2 of extras)
                            nc.tensor.matmul(
                                ps_list[jb][:, sl],
                                extra_sb[0:2, pt * 128:(pt + 1) * 128],
                                rhs_sb[0:2, col],
                                start=False, stop=True)
                    for jb in range(4):
                        # output evict: max(logits + kb + Mneg, -1e8)
                        lr = lrow.tile([128, 1024], F32, tag="lr")
                        nc.vector.tensor_scalar(
                            out=lr[:], in0=ps_list[jb][:], scalar1=NEG_INF,
                            scalar2=None, op0=ALU.max)
                        nc.sync.dma_start(
                            out=out_d[pt * 128:(pt + 1) * 128,
                                      jb * 1024:(jb + 1) * 1024],
                            in_=lr[:])
                    # phase B: label-mask extras + loss activations
                    u_t = upool.tile([128, HW], F32, tag="u")
                    for jb in range(4):
                        ps = ps_list[jb]
                        for half in range(2):
                            sl = slice(half * 512, (half + 1) * 512)
                            col = slice(jb * 1024 + half * 512,
                                        jb * 1024 + half * 512 + 512)
                            nc.tensor.matmul(
                                ps[:, sl],
                                extra_sb[32:49, pt * 128:(pt + 1) * 128],
                                rhs_sb[32:49, col],
                                start=False, stop=True, skip_group_check=True)
                        nc.scalar.activation(
                            u_t[:, jb * 1024:(jb + 1) * 1024], ps[:], AF.Sigmoid)
                    nc.scalar.activation(at_t[:], u_t[:], AF.Arctan,
                                         scale=1.0 / A_EPS)
                    if pt in (1, 3, 5, 7):
                        # sin batches of 2 p-tiles: drains sin work off the
                        # serial ACT tail at the cost of extra table switches
                        for spt in range(pt - 1, pt + 1):
                            for jb in range(4):
                                so = soutp.tile([128, 1024], BF16, tag="so")
                                nc.scalar.activation(
                                    so[:],
                                    at_tiles[spt][:, jb * 1024:(jb + 1) * 1024],
                                    AF.Sin, scale=2.0,
                                    accum_out=s1cols[:, spt * 4 + jb:
                                                     spt * 4 + jb + 1])

                nc.sync.dma_start(out=s_out_d, in_=s1cols[:])
            ctxpool.__exit__(None, None, None)


def _build():
    global _NC
    if _NC is not None:
        return _NC
    nc = bacc.Bacc("TRN2", target_bir_lowering=False, debug=False,
                   num_devices=N_CORES)
    _emit(nc)
    nc.compile()
    _NC = nc
    return nc


def _prep_core(inputs, c):
    b, q = c // 4, c % 4
    r0 = q * ROWS
    p0 = q * P_CORE
    feats = np.ascontiguousarray(inputs["feats"][b], dtype=np.float32)  # [256,64,64]
    fflat = feats.reshape(C, HW)
    masks = np.asarray(inputs["masks"][b], dtype=np.float32).reshape(HW)
    layouts = np.asarray(inputs["layouts"][b]).reshape(HW)

    feats2 = fflat.reshape(2, 128, HW).transpose(1, 0, 2)  # [128,2,4096]

    halo = np.zeros((C, 20, 66), np.float32)
    lo, hi = max(r0 - 2, 0), min(r0 + 18, H)
    halo[:, lo - (r0 - 2):hi - (r0 - 2), 1:65] = feats[:, lo:hi, :]
    fhalo = halo.reshape(2, 128, 20, 66).transpose(1, 0, 2, 3)

    def prep_w33(w):  # [o, i, 3, 3] -> [128, 9, 2, 2, 128]
        t = np.asarray(w, np.float32).transpose(2, 3, 1, 0).reshape(9, 2, 128, 2, 128)
        return np.ascontiguousarray(t.transpose(2, 0, 1, 3, 4))

    wk = np.asarray(inputs["w_kernel"], np.float32).reshape(257, 256)
    wk_t = np.ascontiguousarray(wk.T.reshape(2, 128, 257).transpose(1, 0, 2))
    wf = np.asarray(inputs["w_feats"], np.float32).reshape(256, 256)
    wf_t = np.ascontiguousarray(wf.T.reshape(2, 128, 256).transpose(1, 0, 2))

    bz = np.zeros((128, 11), np.float32)
    bz[:, 0] = inputs["b_pre0"][:128]
    bz[:, 1] = inputs["b_pre0"][128:]
    bz[:, 2] = inputs["b_pre1"][:128]
    bz[:, 3] = inputs["b_pre1"][128:]
    bz[:, 4] = inputs["b_kernel"][:128]
    bz[:, 5] = inputs["b_kernel"][128:256]
    bz[:, 6] = inputs["b_feats"][:128]
    bz[:, 7] = inputs["b_feats"][128:]
    bz[:, 8] = inputs["b_kernel"][256]
    bz[:, 9] = 0.0 if r0 == 0 else 1.0
    bz[:, 10] = 0.0 if r0 + ROWS == H else 1.0

    import ml_dtypes
    oh_hw = (layouts[None, :] == np.arange(16)[:, None]).astype(np.float32)
    lhs_ex = np.ones((17, P_CORE), np.float32)
    lhs_ex[1:17] = oh_hw[:, p0:p0 + P_CORE]
    lhs_ex = lhs_ex.astype(ml_dtypes.bfloat16)
    rhs_ex = np.zeros((19, HW), np.float32)
    rhs_ex[0] = 1.0
    rhs_ex[1] = np.where(masks > 0, 0.0, -BIG)   # Mneg
    rhs_ex[2] = -BIG
    rhs_ex[3:19] = BIG * oh_hw
    rhs_ex = rhs_ex.astype(ml_dtypes.bfloat16)

    return {
        "feats2": np.ascontiguousarray(feats2),
        "fhalo": np.ascontiguousarray(fhalo),
        "wp0": prep_w33(inputs["w_pre0"]),
        "wp1": prep_w33(inputs["w_pre1"]),
        "wk": wk_t, "wf": wf_t, "biases": bz,
        "lhs_ex": lhs_ex, "rhs_ex": rhs_ex,
        "zrow": np.zeros((128, 66), np.float32),
    }


def prep_in_maps(inputs):
    inputs = {k: np.asarray(v) for k, v in inputs.items()}
    return [_prep_core(inputs, c) for c in range(N_CORES)]


def postprocess(results, inputs):
    logits = np.empty((B, HW, HW), np.float32)
    S = np.zeros(B, np.float64)
    masks = np.asarray(inputs["masks"], dtype=np.float32).reshape(B, HW)
    for c in range(N_CORES):
        b, q = c // 4, c % 4
        p0 = q * P_CORE
        logits[b, p0:p0 + P_CORE, :] = results[c]["out"]
        s = results[c]["s_out"].astype(np.float64)          # [128, 32]
        s_vec = s.reshape(128, 8, 4).sum(-1).T.reshape(P_CORE)  # p = pt*128+i
        S[b] += (masks[b, p0:p0 + P_CORE].astype(np.float64) * s_vec).sum()
    S /= A_EPS
    grid = masks.sum(1).astype(np.float64)
    loss = np.float32(((grid * grid - S) / (grid * grid + 1e-5)).mean())
    return logits.reshape(B, HW, H, W), loss


def run(inputs, trace=False):
    nc = _build()
    in_maps = prep_in_maps(inputs)
    res = run_bass_kernel_spmd(nc, in_maps, list(range(N_CORES)), trace=trace)
    logits, loss = postprocess(res.results, inputs)
    return logits, loss, res


def kernel(**inputs):
    logits, loss, _ = run(inputs)
    return logits, loss
